# revision 66
# baseline (speedup 1.0000x reference)
"""Trainium2 Bass kernel for BiDACPI (GAT + CNN + bidirectional attention).

Data-parallel over batch b=16 across 8 NeuronCores (2 graphs per core).
Self-contained: hardcodes all shapes; host-side preprocessing only reshapes /
transposes weights and converts index tensors.

Key optimizations vs the f32r baseline (208.7us -> 151.2us HW):
- bf16 operands everywhere (matmuls get FWL weight loads + full-rate
  small-N; DVE/ACT elementwise halves; DMA bytes halve). The ELU's
  exp(x)-1 keeps exp in f32 to avoid bf16 cancellation near x=0.
- src logits broadcast produced directly in PSUM by a matmul against
  column-replicated (W@a1) weights — removes a copy + gpsimd broadcast
  from every attention block's critical chain.
- attention leaky split across DVE (heads 0-1) and ACT Prelu (rest);
  exp in halves for chunk-level pipelining; zm/ee/U triple-buffered.
- softmax denominators via single-pass reciprocal_approx_fast (~18 bits).
- conv serialized per graph through 2 PSUM banks; freed banks deepen
  attention pipelining (pssq=3, psrow=2 double-buffers srcb).
- startup: dma_start issue slots are ~600ns each, so inputs are packed
  (one row DMA per graph, one critical-weight pack) and the big
  mask/band-matrix streams are split across the scalar/sync queues,
  keeping gpsimd free for the early iota/broadcast chain.
"""
import numpy as np

import concourse.bass as bass
import concourse.mybir as mybir
import concourse.tile as tile
from concourse import bacc

F32 = mybir.dt.float32
BF16 = mybir.dt.bfloat16
I32 = mybir.dt.int32
AT = mybir.AluOpType
AF = mybir.ActivationFunctionType

# Problem constants
B = 16
NCORES = 8
G = B // NCORES          # graphs per core
N = 512                  # atoms per graph
L = 1024                 # amino length
CD = 128                 # comp_dim
PD = 128                 # prot_dim
GD = 64                  # gat_dim
H = 4                    # heads
LAT = 128                # latent
NA = 100                 # num_atom
NAM = 30                 # num_amino
LC = 3                   # conv layers
KW = 11                  # conv kernel width
ALPHA = 0.2
MASKNEG = -1.0e30
NT = N // 128            # 4 j-chunks
PADL = KW // 2
GAT_BF16 = True          # bf16 operands in the GAT/attention path


def build_core_program(debug=False, sim_bf16=False):
    """Build the per-core SPMD program (identical across cores).

    debug=True builds the CoreSim-compatible variant (no Prelu — the sim
    lacks it; f32 dtypes unless sim_bf16, which keeps MD=BF16 to bisect
    bf16 numerics in the simulator).
    """
    if debug:
        nc = bacc.Bacc(None, target_bir_lowering=False, debug=True)
    else:
        nc = bacc.Bacc(None)
    F32R = mybir.dt.float32r
    if debug:
        MD = BF16 if sim_bf16 else F32
        CT = MD
    else:
        MD = BF16 if GAT_BF16 else F32R  # GAT operand dtype
        CT = MD                          # conv-path operand dtype
    use_prelu = not debug
    abufs = 2 if debug else 3   # attention tile pipelining depth

    # ---- DRAM I/O ----
    d_rows = nc.dram_tensor("rows_packed", [G, 2 * N + 2 * L], MD,
                            kind="ExternalInput")
    # ladjT_r[g, p, t, i] = additive mask for edge j->i  (j = t*128+p)
    d_ladjT = nc.dram_tensor("ladjT_r", [G, 128, NT, N], MD,
                             kind="ExternalInput")
    # packed critical weights: [E_atom_pad | W_gat_r | Wa1rep]
    d_wcrit = nc.dram_tensor("wcrit", [128, CD + H * GD + H * 128], MD,
                             kind="ExternalInput")
    d_Eam = nc.dram_tensor("E_amino", [NAM, PD], CT, kind="ExternalInput")
    # a2_rows[0, h, q] = a_gat[h, GD+q]; a2go_row[0, q] = a_go[CD+q]
    d_a2r = nc.dram_tensor("a2_rows", [1, H, GD], MD, kind="ExternalInput")
    d_a2go = nc.dram_tensor("a2go_row", [1, CD], MD, kind="ExternalInput")
    # W_go_r[p, c, q] = W_go[c*128+p, q]
    d_Wgo = nc.dram_tensor("W_go_r", [128, 2, CD], MD, kind="ExternalInput")
    # Wgoa1rep[p, c, q] = (W_go @ a1_go)[c*128+p] for all q
    d_Wgoa1rep = nc.dram_tensor("Wgoa1rep", [128, 2, 128], MD,
                                kind="ExternalInput")
    d_Wc = nc.dram_tensor("W_comp_wT", [CD, LAT], MD, kind="ExternalInput")
    d_bc = nc.dram_tensor("W_comp_b", [LAT, 1], F32, kind="ExternalInput")
    # MiT_r[p, l, i, q] = band matrix MiT[l, i, p, q]
    d_MiT = nc.dram_tensor("MiT_r", [PD, LC, KW, PD], CT,
                           kind="ExternalInput")
    d_cb = nc.dram_tensor("conv_b", [LC, 1], F32, kind="ExternalInput")
    d_Wa = nc.dram_tensor("W_att_wT", [LAT, LAT], MD, kind="ExternalInput")
    d_ba = nc.dram_tensor("W_att_b", [LAT, 1], F32, kind="ExternalInput")
    d_pw = nc.dram_tensor("pw_cols", [LAT, 2], F32, kind="ExternalInput")
    d_pb = nc.dram_tensor("pred_b", [1, 1], F32, kind="ExternalInput")
    # const_oz[:, :PADL] = 0.0, const_oz[:, PADL] = 1.0 (MD dtype)
    d_oz = nc.dram_tensor("const_oz", [128, PADL + 1], MD, kind="ExternalInput")
    d_out = nc.dram_tensor("out", [G, 1], F32, kind="ExternalOutput")

    with tile.TileContext(nc) as tc:
        with (
            tc.tile_pool(name="const", bufs=1) as cpool,
            tc.tile_pool(name="work", bufs=1) as wpool,
            tc.tile_pool(name="big", bufs=2) as bpool,
            tc.tile_pool(name="adj", bufs=2) as apool,
            tc.tile_pool(name="rows", bufs=1) as rpool,
            tc.tile_pool(name="ps_sq", bufs=3, space="PSUM") as pssq,
            tc.tile_pool(name="ps_row", bufs=2, space="PSUM") as psrow,
            tc.tile_pool(name="ps_cv", bufs=2, space="PSUM") as pscv,
            tc.tile_pool(name="ps_wh", bufs=1, space="PSUM") as pswh,
        ):
            # ---- per-graph inputs first (critical path) ----
            # one packed row DMA per graph: [atoms | amask | amino | pmask]
            g_in = []
            for g in range(G):
                rowt = rpool.tile([1, 2 * N + 2 * L], MD, tag="ginrows",
                                  bufs=2, name="rowt")
                nc.sync.dma_start(out=rowt, in_=d_rows[g : g + 1, :])
                arow = rowt[:, 0:N]
                amrow = rowt[:, N : 2 * N]
                prow = rowt[:, 2 * N : 2 * N + L]
                pmrow = rowt[:, 2 * N + L :]
                ladjT = [apool.tile([128, N], MD, tag=f"ladjT{t}",
                                    name=f"ladjT{t}") for t in range(NT)]
                g_in.append((arow, prow, ladjT, amrow, pmrow))

            # ---- constants / weights resident in SBUF ----
            ioi = cpool.tile([128, 1], I32)
            nc.gpsimd.iota(ioi, pattern=[[0, 1]], base=0, channel_multiplier=1)
            iofc = cpool.tile([128, 1], F32)
            nc.vector.tensor_copy(iofc, ioi)
            ones_col = cpool.tile([128, 1], F32)
            nc.vector.memset(ones_col, 1.0)
            ozsb = cpool.tile([128, PADL + 1], MD)
            nc.sync.dma_start(out=ozsb, in_=d_oz[:, :])
            ones_col_m = ozsb[:, PADL : PADL + 1]

            wcrit = cpool.tile([128, CD + H * GD + H * 128], MD)
            nc.sync.dma_start(out=wcrit, in_=d_wcrit[:, :])
            Eat = wcrit[:, 0:CD]
            Wg = wcrit[:, CD : CD + H * GD]
            Wa1rep = wcrit[:, CD + H * GD :]
            Eam = cpool.tile([NAM, PD], CT)
            nc.sync.dma_start(out=Eam, in_=d_Eam[:, :])
            a2r = cpool.tile([1, H, GD], MD)
            nc.sync.dma_start(out=a2r, in_=d_a2r[:, :, :])
            # mask DMAs after the critical weights, split across queues
            qrot = [nc.scalar, nc.sync]
            for g in range(G):
                for t in range(NT):
                    qrot[(g * NT + t) % 2].dma_start(
                        out=g_in[g][2][t], in_=d_ladjT[g, :, t, :])
            # ---- remaining weights ----
            a2go = cpool.tile([1, CD], MD)
            nc.sync.dma_start(out=a2go, in_=d_a2go[:, :])
            Wgo = cpool.tile([128, 2, CD], MD)
            nc.sync.dma_start(out=Wgo, in_=d_Wgo[:, :, :])
            Wgoa1rep = cpool.tile([128, 2, 128], MD)
            nc.sync.dma_start(out=Wgoa1rep, in_=d_Wgoa1rep[:, :, :])
            Wc = cpool.tile([CD, LAT], MD)
            nc.sync.dma_start(out=Wc, in_=d_Wc[:, :])
            bc = cpool.tile([LAT, 1], F32)
            nc.sync.dma_start(out=bc, in_=d_bc[:, :])
            cb = cpool.tile([128, LC], F32)
            nc.sync.dma_start(
                out=cb,
                in_=bass.AP(tensor=d_cb, offset=0, ap=[[0, 128], [1, LC], [0, 1]]),
            )
            Wa = cpool.tile([LAT, LAT], MD)
            nc.scalar.dma_start(out=Wa, in_=d_Wa[:, :])
            ba = cpool.tile([LAT, 1], F32)
            nc.sync.dma_start(out=ba, in_=d_ba[:, :])
            pw = cpool.tile([LAT, 2], F32)
            nc.sync.dma_start(out=pw, in_=d_pw[:, :])
            pb = cpool.tile([1, 1], F32)
            nc.sync.dma_start(out=pb, in_=d_pb[:, :])

            MiT = []
            for lyr, qe in zip(range(LC), (nc.scalar, nc.sync, nc.scalar)):
                mt = cpool.tile([PD, KW, PD], CT)
                qe.dma_start(out=mt, in_=d_MiT[:, lyr, :, :])
                MiT.append(mt)
            def leaky(out, in_, alpha, bias=None):
                """out = leaky_relu(in_ + bias, alpha). in_ may be PSUM."""
                if use_prelu:
                    if bias is None:
                        nc.scalar.activation(out=out, in_=in_, func=AF.Prelu,
                                             alpha=alpha)
                    else:
                        nc.scalar.activation(out=out, in_=in_, func=AF.Prelu,
                                             bias=bias, alpha=alpha)
                    return
                src = in_
                if bias is not None:
                    t = wpool.tile(list(out.shape), F32, tag="t2k",
                                   bufs=8, name="lkb")
                    nc.scalar.activation(out=t, in_=in_, func=AF.Identity,
                                         bias=bias)
                    src = t
                nc.vector.scalar_tensor_tensor(
                    out=out, in0=src, scalar=alpha, in1=src,
                    op0=AT.mult, op1=AT.max)

            def elu_into(out_ap, hp_src, rb, m, tag_sfx):
                """out = elu(hp_src * rb); hp_src PSUM (m, N), rb SBUF (m, N)."""
                hpn = wpool.tile([m, N], MD, tag="t2k", bufs=8,
                                 name="hpn" + tag_sfx)
                nc.vector.scalar_tensor_tensor(
                    out=hpn, in0=hp_src, scalar=1.0, in1=rb,
                    op0=AT.mult, op1=AT.mult)
                xm = wpool.tile([m, N], MD, tag="t2k", bufs=8,
                                name="xm" + tag_sfx)
                nc.vector.tensor_scalar(out=xm, in0=hpn, scalar1=0.0,
                                        scalar2=None, op0=AT.min)
                em = wpool.tile([m, N], F32, tag="emx", bufs=3,
                                name="em" + tag_sfx)
                nc.scalar.activation(out=em, in_=xm, func=AF.Exp)
                nc.vector.scalar_tensor_tensor(
                    out=out_ap, in0=em, scalar=-1.0, in1=hpn,
                    op0=AT.add, op1=AT.max)

            # per-graph state carried into the fused conv / tail phases
            st = [dict() for _ in range(G)]

            for g in range(G):
                # ---------- atom embeddings via one-hot matmul ----------
                arow, prow, ladjT, amrow, pmrow = g_in[g]
                ab = wpool.tile([128, N], MD, tag="t2k", bufs=8, name="ab")
                nc.gpsimd.partition_broadcast(ab, arow)
                ohA = wpool.tile([128, N], MD, tag="t2k", bufs=8, name="ohA")
                nc.vector.tensor_scalar(out=ohA, in0=ab, scalar1=iofc,
                                        scalar2=None, op0=AT.is_equal)
                avT_ps = pssq.tile([128, N], F32, tag="mm_sq", name="avT_ps")
                nc.tensor.matmul(avT_ps, Eat, ohA, start=True, stop=True)
                avT = wpool.tile([128, N], MD, tag="avT", bufs=2, name="avT")
                nc.scalar.copy(avT, avT_ps)

                m01 = wpool.tile([128, N], MD, tag="multi01", bufs=2, name="m01")
                m23 = wpool.tile([128, N], MD, tag="multi23", bufs=2, name="m23")
                multi = [m01, m23]

                def gat_attention(whsb, nk, srcb_ps, dcol, rowsum_sep,
                                  ladjT, leaky_act, tag_pfx):
                    """z -> leaky -> exp -> hp (+rowsum) -> 1/rowsum bcast.

                    srcb_ps: PSUM (128, N) f32, src value replicated across
                    partitions; dcol: SBUF (128, NT) f32 dst columns.
                    Returns (hp_ps, rb).
                    """
                    # z = src_bcast + dst + ladj  (one fused DVE op per chunk)
                    zm = bpool.tile([128, NT, N], MD, tag="zm", bufs=abufs,
                                    name="zm")
                    for t in range(NT):
                        nc.vector.scalar_tensor_tensor(
                            out=zm[:, t, :], in0=srcb_ps,
                            scalar=dcol[:, t : t + 1],
                            in1=ladjT[t], op0=AT.add, op1=AT.add)
                    ee = bpool.tile([128, NT, N], MD, tag="ee", bufs=abufs,
                                    name="ee")
                    U = bpool.tile([128, NT, N], MD, tag="U", bufs=abufs,
                                   name="U")
                    if leaky_act and use_prelu:
                        nc.scalar.activation(out=ee, in_=zm,
                                             func=AF.Prelu, alpha=ALPHA)
                    else:
                        nc.vector.scalar_tensor_tensor(
                            out=ee, in0=zm, scalar=ALPHA, in1=zm,
                            op0=AT.mult, op1=AT.max)
                    for hf in range(2):
                        sl = slice(hf * (NT // 2), (hf + 1) * (NT // 2))
                        nc.scalar.activation(out=U[:, sl, :], in_=ee[:, sl, :],
                                             func=AF.Exp)
                    hp_m = nk + (0 if rowsum_sep else 1)
                    hp_ps = pssq.tile([128, N], F32, tag="mm_sq", name="hp_ps")
                    for t in range(NT):
                        nc.tensor.matmul(hp_ps[:hp_m, :],
                                         whsb[:, t, :], U[:, t, :],
                                         start=(t == 0), stop=(t == NT - 1))
                    if rowsum_sep:
                        rs_ps = psrow.tile([1, N], F32, tag="ps_row",
                                           name="rs_ps")
                        for t in range(NT):
                            nc.tensor.matmul(rs_ps, ones_col_m, U[:, t, :],
                                             start=(t == 0), stop=(t == NT - 1))
                        zrow = rs_ps
                    else:
                        zrow = hp_ps[nk : nk + 1, :]
                    zrw = rpool.tile([1, N], F32, tag="r2k", bufs=4,
                                     name="zrw")
                    nc.scalar.copy(zrw, zrow)
                    rinv = rpool.tile([1, N], F32, tag="r2k", bufs=4,
                                      name="rinv")
                    nc.vector.reciprocal_approx_fast(out=rinv, in_=zrw)
                    rb = wpool.tile([nk, N], F32, tag="rbx", bufs=3,
                                    name="rb" + tag_pfx)
                    nc.gpsimd.partition_broadcast(rb, rinv)
                    return hp_ps, rb

                # ---------- GAT heads ----------
                # all heads' Wh chunks, batched so each avT chunk (lhsT) is
                # loaded once; one N=256 matmul per chunk covers all 4 heads
                whsb_all = wpool.tile([128, NT, H, GD + 1], MD,
                                      tag="whsb_all", bufs=1, name="whsb_all")
                nc.scalar.copy(
                    out=whsb_all[:, :, :, GD : GD + 1],
                    in_=bass.AP(tensor=ozsb.tensor,
                                offset=ozsb.offset + PADL,
                                ap=[ozsb.ap[0], [0, NT], [0, H], [0, 1]]))
                for half in range(2):
                    wh_all = pswh.tile([128, 2, H, GD], F32, tag="wh_all",
                                       name="wh_all")
                    for t2 in range(2):
                        t = half * 2 + t2
                        nc.tensor.matmul(
                            wh_all[:, t2, :, :],
                            avT[:, t * 128 : (t + 1) * 128],
                            Wg, start=True, stop=True)
                    nc.scalar.copy(
                        whsb_all[:, half * 2 : half * 2 + 2, :, :GD], wh_all)

                for h in range(H):
                    whsb = whsb_all[:, :, h, :]
                    srcb_ps = psrow.tile([128, N], F32, tag="ps_row",
                                         name="srcb_ps")
                    nc.tensor.matmul(srcb_ps, Wa1rep[:, h * 128 : (h + 1) * 128], avT,
                                     start=True, stop=True)
                    a2b = wpool.tile([128, GD], MD, tag="a2b", bufs=5,
                                     name="a2b")
                    nc.gpsimd.partition_broadcast(a2b, a2r[0:1, h, :])
                    dcol = rpool.tile([128, NT], F32, tag="dcol", bufs=5,
                                      name="dcol")
                    dsc = rpool.tile([128, GD], MD, tag="dsc", bufs=4,
                                     name="dsc")
                    for t in range(NT):
                        nc.vector.scalar_tensor_tensor(
                            out=dsc, in0=whsb[:, t, :GD], scalar=1.0,
                            in1=a2b, op0=AT.mult, op1=AT.mult,
                            accum_out=dcol[:, t : t + 1])
                    hp_ps, rb = gat_attention(whsb, GD, srcb_ps,
                                              dcol, False, ladjT,
                                              h >= 2, "h")
                    dsti = multi[h // 2]
                    off = (h % 2) * GD
                    elu_into(dsti[off : off + GD, :], hp_ps[:GD, :], rb,
                             GD, "h")

                # ---------- GAT output layer ----------
                wh2_ps = pssq.tile([128, NT, CD], F32, tag="mm_sq",
                                   name="wh2_ps")
                for t in range(NT):
                    for c in range(2):
                        nc.tensor.matmul(
                            wh2_ps[:, t, :],
                            multi[c][:, t * 128 : (t + 1) * 128],
                            Wgo[:, c, :], start=(c == 0), stop=(c == 1))
                wh2sb = wpool.tile([128, NT, CD], MD, tag="t2k", bufs=8,
                                   name="wh2sb")
                nc.scalar.copy(wh2sb, wh2_ps)

                srcb2_ps = psrow.tile([128, N], F32, tag="ps_row",
                                      name="srcb2_ps")
                for c in range(2):
                    nc.tensor.matmul(srcb2_ps, Wgoa1rep[:, c, :], multi[c],
                                     start=(c == 0), stop=(c == 1))
                a2gob = wpool.tile([128, CD], MD, tag="a2b", bufs=5,
                                   name="a2gob")
                nc.gpsimd.partition_broadcast(a2gob, a2go)
                dcol2 = rpool.tile([128, NT], F32, tag="dcol", bufs=5,
                                   name="dcol2")
                dsc2 = rpool.tile([128, CD], MD, tag="dsc", bufs=4,
                                  name="dsc2")
                for t in range(NT):
                    nc.vector.scalar_tensor_tensor(
                        out=dsc2, in0=wh2sb[:, t, :], scalar=1.0,
                        in1=a2gob, op0=AT.mult, op1=AT.mult,
                        accum_out=dcol2[:, t : t + 1])
                hp2_ps, rb2 = gat_attention(wh2sb, CD, srcb2_ps, dcol2,
                                            True, ladjT, True, "o")
                xT = wpool.tile([CD, N], MD, tag="xT", bufs=2, name="xT")
                elu_into(xT, hp2_ps, rb2, CD, "o")

                # ---------- atoms_vec -> a_v -> comp pooling ----------
                av_ps = pssq.tile([LAT, N], F32, tag="mm_sq", name="av_ps")
                nc.tensor.matmul(av_ps, Wc, xT, start=True, stop=True)
                avec = wpool.tile([LAT, N], MD, tag="t2k", bufs=8, name="avec")
                leaky(avec, av_ps, ALPHA, bias=bc)
                av2_ps = pssq.tile([LAT, N], F32, tag="mm_sq", name="av2_ps")
                nc.tensor.matmul(av2_ps, Wa, avec, start=True, stop=True)
                a_v = wpool.tile([LAT, N], MD, tag="t2k", bufs=8, name="a_v")
                leaky(a_v, av2_ps, ALPHA, bias=ba)

                amb = wpool.tile([128, N], MD, tag="t2k", bufs=8, name="amb")
                nc.gpsimd.partition_broadcast(amb, amrow)
                cscr = wpool.tile([LAT, N], MD, tag="t2k", bufs=8, name="cscr")
                comp_acc = rpool.tile([LAT, 1], F32, tag="c1", bufs=4,
                                      name="comp_acc")
                nc.vector.scalar_tensor_tensor(
                    out=cscr, in0=a_v, scalar=1.0, in1=amb,
                    op0=AT.mult, op1=AT.mult, accum_out=comp_acc)
                amscr = wpool.tile([128, N], MD, tag="t2k", bufs=8,
                                   name="amscr")
                amsum = rpool.tile([128, 1], F32, tag="c2", bufs=8,
                                   name="amsum")
                nc.vector.tensor_scalar(out=amscr, in0=amb, scalar1=1.0,
                                        scalar2=0.0, op0=AT.mult,
                                        op1=AT.add, accum_out=amsum)
                amr = rpool.tile([128, 1], F32, tag="c2", bufs=8, name="amr")
                nc.vector.reciprocal(amr, amsum)
                cp = rpool.tile([128, 2], F32, tag="cp", bufs=6, name="cp")
                nc.vector.tensor_scalar(out=cp[:, 0:1], in0=comp_acc,
                                        scalar1=amr, scalar2=None,
                                        op0=AT.mult)
                st[g]["cp"] = cp

                # ---------- protein embedding (conv input) ----------
                pbm = wpool.tile([128, L], MD, tag="t4k", bufs=3, name="pbm")
                nc.gpsimd.partition_broadcast(pbm, prow)
                ohP = wpool.tile([NAM, L], CT, tag="t4k", bufs=3, name="ohP")
                nc.vector.tensor_scalar(out=ohP, in0=pbm[:NAM, :],
                                        scalar1=iofc[:NAM, :], scalar2=None,
                                        op0=AT.is_equal)
                pv = bpool.tile([PD, L + 2 * PADL], CT, tag=f"pv{g}_0",
                                bufs=1, name="pv")
                nc.scalar.copy(out=pv[:, :PADL], in_=ozsb[:, :PADL])
                nc.scalar.copy(out=pv[:, PADL + L :], in_=ozsb[:, :PADL])
                for nn in range(2):
                    pvT_ps = pscv.tile([PD, 512], F32, tag="ps_cv",
                                       name="pvT_ps")
                    nc.tensor.matmul(pvT_ps, Eam,
                                     ohP[:, nn * 512 : (nn + 1) * 512],
                                     start=True, stop=True)
                    nc.scalar.copy(
                        pv[:, PADL + nn * 512 : PADL + (nn + 1) * 512], pvT_ps)
                st[g]["pv"] = pv

            # ---------- conv layers, both graphs interleaved ----------
            # (shared MiT weights stay loaded across 4 consecutive matmuls)
            for lyr in range(LC):
                for g in range(G):
                    pvo = bpool.tile([PD, L + 2 * PADL], CT,
                                     tag=f"pv{g}_{1 - lyr % 2}", bufs=1,
                                     name="pvo")
                    nc.scalar.copy(out=pvo[:, :PADL], in_=ozsb[:, :PADL])
                    nc.scalar.copy(out=pvo[:, PADL + L :],
                                   in_=ozsb[:, :PADL])
                    pv = st[g]["pv"]
                    cv_ps = [pscv.tile([PD, 512], F32, tag="ps_cv",
                                       name=f"cv{g}{nn}") for nn in range(2)]
                    for i in range(KW):
                        for nn in range(2):
                            nc.tensor.matmul(
                                cv_ps[nn], MiT[lyr][:, i, :],
                                pv[:, nn * 512 + i : nn * 512 + i + 512],
                                start=(i == 0), stop=(i == KW - 1))
                    for nn in range(2):
                        nc.scalar.activation(
                            out=pvo[:, PADL + nn * 512 :
                                    PADL + (nn + 1) * 512],
                            in_=cv_ps[nn], func=AF.Relu,
                            bias=cb[:, lyr : lyr + 1])
                    st[g]["pv"] = pvo

            # ---------- p_v + prot pooling + head, per graph ----------
            for g in range(G):
                amv = st[g]["pv"][:, PADL : PADL + L]
                cp = st[g]["cp"]
                pmrow = g_in[g][4]
                p_v = wpool.tile([LAT, L], MD, tag="t4k", bufs=3,
                                 name="p_v")
                for nn in range(2):
                    pv_ps = pscv.tile([LAT, 512], F32, tag="ps_cv",
                                      name="pv_ps")
                    nc.tensor.matmul(pv_ps, Wa,
                                     amv[:, nn * 512 : (nn + 1) * 512],
                                     start=True, stop=True)
                    leaky(p_v[:, nn * 512 : (nn + 1) * 512], pv_ps, ALPHA,
                          bias=ba)
                pmb = wpool.tile([128, L], MD, tag="t4k", bufs=3, name="pmb")
                nc.gpsimd.partition_broadcast(pmb, pmrow)
                pscr = wpool.tile([LAT, L], MD, tag="t4k", bufs=3, name="pscr")
                prot_acc = rpool.tile([LAT, 1], F32, tag="c1", bufs=4,
                                      name="prot_acc")
                nc.vector.scalar_tensor_tensor(
                    out=pscr, in0=p_v, scalar=1.0, in1=pmb,
                    op0=AT.mult, op1=AT.mult, accum_out=prot_acc)
                pmscr = wpool.tile([128, L], MD, tag="t4k", bufs=3,
                                   name="pmscr")
                pmsum = rpool.tile([128, 1], F32, tag="c2", bufs=8,
                                   name="pmsum")
                nc.vector.tensor_scalar(out=pmscr, in0=pmb, scalar1=1.0,
                                        scalar2=0.0, op0=AT.mult,
                                        op1=AT.add, accum_out=pmsum)
                pmr = rpool.tile([128, 1], F32, tag="c2", bufs=8, name="pmr")
                nc.vector.reciprocal(pmr, pmsum)
                nc.vector.tensor_scalar(out=cp[:, 1:2], in0=prot_acc,
                                        scalar1=pmr, scalar2=None,
                                        op0=AT.mult)

                lr2 = rpool.tile([128, 2], F32, tag="cp", bufs=6, name="lr2")
                leaky(lr2, cp, ALPHA * ALPHA)
                dscr = rpool.tile([128, 2], F32, tag="cp", bufs=6, name="dscr")
                dacc = rpool.tile([128, 1], F32, tag="c1", bufs=4, name="dacc")
                nc.vector.scalar_tensor_tensor(
                    out=dscr, in0=lr2, scalar=1.0, in1=pw,
                    op0=AT.mult, op1=AT.mult, accum_out=dacc)
                fin_ps = psrow.tile([1, 1], F32, tag="ps_row", name="fin_ps")
                nc.tensor.matmul(fin_ps, dacc, ones_col,
                                 start=True, stop=True)
                res = rpool.tile([1, 1], F32, tag="c2", bufs=8, name="res")
                nc.scalar.activation(out=res, in_=fin_ps, func=AF.Identity,
                                     bias=pb)
                nc.sync.dma_start(out=d_out[g : g + 1, :], in_=res)

    return nc


def preprocess(inputs, md_bf16=True):
    """Host-side prep: shard over cores, transpose/reshape weights."""
    import ml_dtypes
    md = ml_dtypes.bfloat16 if (md_bf16 and GAT_BF16) else np.float32
    atoms = np.asarray(inputs["atoms"]).astype(np.float32)
    atoms_mask = np.asarray(inputs["atoms_mask"]).astype(np.float32)
    adjacency = np.asarray(inputs["adjacency"])
    amino = np.asarray(inputs["amino"]).astype(np.float32)
    amino_mask = np.asarray(inputs["amino_mask"]).astype(np.float32)
    E_atom = np.asarray(inputs["E_atom"]).astype(np.float32)
    E_amino = np.asarray(inputs["E_amino"]).astype(np.float32)
    W_gat = np.asarray(inputs["W_gat"]).astype(np.float32)
    a_gat = np.asarray(inputs["a_gat"]).astype(np.float32)
    W_go = np.asarray(inputs["W_go"]).astype(np.float32)
    a_go = np.asarray(inputs["a_go"]).astype(np.float32)
    W_comp_w = np.asarray(inputs["W_comp_w"]).astype(np.float32)
    W_comp_b = np.asarray(inputs["W_comp_b"]).astype(np.float32)
    conv_w = np.asarray(inputs["conv_w"]).astype(np.float32)
    conv_b = np.asarray(inputs["conv_b"]).astype(np.float32)
    W_att_w = np.asarray(inputs["W_att_w"]).astype(np.float32)
    W_att_b = np.asarray(inputs["W_att_b"]).astype(np.float32)
    pred_w = np.asarray(inputs["pred_w"]).astype(np.float32)
    pred_b = np.asarray(inputs["pred_b"]).astype(np.float32)

    # additive mask, transposed, pre-tiled: [g, p, t, i] = mask(j=t*128+p, i)
    ladjT = np.where(adjacency.transpose(0, 2, 1) > 0, np.float32(0.0),
                     np.float32(MASKNEG)).astype(np.float32)
    ladjT_r = np.ascontiguousarray(
        ladjT.reshape(B, NT, 128, N).transpose(0, 2, 1, 3))

    E_atom_pad = np.zeros((128, CD), np.float32)
    E_atom_pad[:NA] = E_atom

    # conv band matrices: MiT[l, i, din, dout] = conv_w[l,0,0,i, din-dout+5]
    MiT = np.zeros((LC, KW, PD, PD), np.float32)
    din = np.arange(PD)[:, None]
    dout = np.arange(PD)[None, :]
    v = din - dout + (KW // 2)
    valid = (v >= 0) & (v < KW)
    vc = np.clip(v, 0, KW - 1)
    for lyr in range(LC):
        for i in range(KW):
            MiT[lyr, i] = np.where(valid, conv_w[lyr, 0, 0, i, vc], 0.0)
    MiT_r = np.ascontiguousarray(MiT.transpose(2, 0, 1, 3))

    W_gat_r = np.ascontiguousarray(W_gat.transpose(1, 0, 2))
    # Wa1rep[p, h, q] = (W_gat[h] @ a1_h)[p]  (replicated over q)
    Wa1 = np.einsum("hpq,hq->ph", W_gat, a_gat[:, :GD])  # (CD, H)
    Wa1rep = np.ascontiguousarray(
        np.repeat(Wa1[:, :, None], 128, axis=2))
    W_go_r = np.ascontiguousarray(
        W_go.reshape(2, 128, CD).transpose(1, 0, 2))
    Wgoa1 = (W_go @ a_go[:CD]).reshape(2, 128)  # (c, p)
    Wgoa1rep = np.ascontiguousarray(
        np.repeat(Wgoa1.transpose(1, 0)[:, :, None], 128, axis=2))

    oz = np.concatenate([np.zeros((128, PADL), np.float32),
                         np.ones((128, 1), np.float32)], axis=1)

    shared = {
        "wcrit": np.ascontiguousarray(np.concatenate(
            [E_atom_pad, W_gat_r.reshape(128, H * GD),
             Wa1rep.reshape(128, H * 128)], axis=1)).astype(md),
        "E_amino": np.ascontiguousarray(E_amino).astype(md),
        "a2_rows": np.ascontiguousarray(a_gat[:, GD:][None, :, :]).astype(md),
        "a2go_row": np.ascontiguousarray(a_go[CD:][None, :]).astype(md),
        "W_go_r": W_go_r.astype(md),
        "Wgoa1rep": Wgoa1rep.astype(md),
        "W_comp_wT": np.ascontiguousarray(W_comp_w.T).astype(md),
        "W_comp_b": np.ascontiguousarray(W_comp_b[:, None]),
        "MiT_r": MiT_r.astype(md),
        "conv_b": np.ascontiguousarray(conv_b.reshape(LC, 1)),
        "W_att_wT": np.ascontiguousarray(W_att_w.T).astype(md),
        "W_att_b": np.ascontiguousarray(W_att_b[:, None]),
        "pw_cols": np.ascontiguousarray(
            np.stack([pred_w[0, :LAT], pred_w[0, LAT:]], axis=1)),
        "pred_b": np.ascontiguousarray(pred_b.reshape(1, 1)),
        "const_oz": np.ascontiguousarray(oz).astype(md),
    }
    in_maps = []
    for c in range(NCORES):
        sl = slice(c * G, (c + 1) * G)
        m = dict(shared)
        m["rows_packed"] = np.ascontiguousarray(np.concatenate(
            [atoms[sl], atoms_mask[sl], amino[sl], amino_mask[sl]],
            axis=1)).astype(md)
        m["ladjT_r"] = np.ascontiguousarray(ladjT_r[sl]).astype(md)
        in_maps.append(m)
    return in_maps


_CACHED_NC = None


def kernel(**inputs) -> np.ndarray:
    global _CACHED_NC
    from concourse.bass_utils import run_bass_kernel_spmd

    if _CACHED_NC is None:
        nc = build_core_program()
        nc.finalize()
        _CACHED_NC = nc
    nc = _CACHED_NC
    in_maps = preprocess(inputs)
    res = run_bass_kernel_spmd(nc, in_maps, core_ids=list(range(NCORES)))
    out = np.concatenate([res.results[c]["out"] for c in range(NCORES)], axis=0)
    return out.astype(np.float32)
